# revision 1
# baseline (speedup 1.0000x reference)
"""CFConv (SchNet continuous-filter convolution) on 8 Trainium2 NeuronCores.

Reference computation (per atom i, neighbor slot k):
    W[i,k,:]  = ssp(dRexp[i,k,:] @ W1 + b1) @ W2 + b2       (filter network)
    C[i,k]    = (dR[i,k] <= 5.0)                            (hard cutoff)
    y         = x @ W_in2f                                  (atom embeddings)
    out[i,:]  = ssp( sum_k C*mask*W[i,k,:]*y[nbh[i,k],:] @ W_f2out + b_f2out )
    where ssp(v) = softplus(v) - log(2)

Sharding: atoms split across 8 cores (1250 each, padded to 1280).  Every core
builds the full y embedding table [10112, 128] locally (cheap: one 10112x128
@ 128x128 matmul) and writes it to its own DRAM; the neighbor gather is then a
purely local indirect DMA.  The hard cutoff and pairwise mask are folded into
the gather indices on the host: masked edges gather a guaranteed-zero row of
the y table, so no mask/cutoff work happens on device.

Device layout choices:
  - filter net runs feature-major: h1^T [f=128, e] tiles with W1 as the
    stationary matmul operand (streaming edges on the free dim)
  - mm2 runs per 128-edge tile with h1s^T as lhsT producing W [e, h] directly
    in the same edge-on-partition layout the gather output uses
  - edges are ordered slot-major (e = k*1280 + i) so each 128-edge tile is
    128 atoms at a fixed neighbor slot k; summing over k is then plain
    tile accumulation
  - ssp(v) = softplus(v) - log2 is computed as ln(0.5*exp(v) + 0.5) with two
    scalar-engine ops from the shared exp/ln ACT table set (trn2 has no
    softplus table; this form folds the -log2 for free)
  - the neighbor gather uses the gpsimd dma_gather custom op (128*kk rows per
    atom tile, single_packet=False); 4 SWDGE queues overlap descriptor
    generation with gather execution across tiles
  - each atom's neighbor slots are host-sorted valid-first and the slot axis
    truncated to the max valid count over the dataset (rounded to a quad,
    kk=36 for this input: the uniform dR cutoff masks ~50% of edges), cutting
    gather generation/DMA and all per-edge compute by kk/K
"""

import numpy as np
from contextlib import ExitStack

import concourse.bass as bass
import concourse.bacc as bacc
import concourse.mybir as mybir
import concourse.tile as tile
from concourse.masks import make_identity

F32 = mybir.dt.float32
I32 = mybir.dt.int32
I16 = mybir.dt.int16
AOP = mybir.AluOpType
ACTF = mybir.ActivationFunctionType

# ---- geometry (hardcoded for nn_CFConv_13245679141058) ----
N_ATOMS = 10000
K = 48                    # neighbors per atom
NIN = NF = NOUT = 128
NG = 25                   # gaussians
NCORES = 8
A_CORE = N_ATOMS // NCORES        # 1250 real atoms per core
A_PAD = 1280                      # padded to 10 tiles of 128
NT = A_PAD // 128                 # 10 atom tiles per core
YROWS = 10112                     # y table rows (79 tiles of 128); rows >= 10000 are zero
YT_TILES = YROWS // 128           # 79
ZIDX = N_ATOMS                    # guaranteed-zero row index used by masked edges
CHUNK = 256                       # edges (atoms at fixed k) per mm1 chunk
NCHUNK = A_PAD // CHUNK           # 5
R_CUTOFF = 5.0
LOG2 = float(np.log(2.0))


def build_nc(debug=False, kk=K):
    nc = bacc.Bacc(num_swdge_queues=4)

    # --- per-core external inputs ---
    xT_d = nc.declare_dram_parameter("xT", [NIN, YROWS], F32, isOutput=False)
    w_in2f_d = nc.declare_dram_parameter("w_in2f", [NIN, NF], F32, isOutput=False)
    w1_d = nc.declare_dram_parameter("w1", [NG, NF], F32, isOutput=False)
    w2_d = nc.declare_dram_parameter("w2", [NF, NF], F32, isOutput=False)
    wf_d = nc.declare_dram_parameter("wf", [NF, NOUT], F32, isOutput=False)
    b1_d = nc.declare_dram_parameter("b1", [NF, 1], F32, isOutput=False)
    b2_d = nc.declare_dram_parameter("b2", [1, NF], F32, isOutput=False)
    bf_d = nc.declare_dram_parameter("bf", [NOUT, 1], F32, isOutput=False)
    dRexpT_d = nc.declare_dram_parameter("dRexpT", [NG, kk * A_PAD], F32, isOutput=False)
    # dma_gather index table: per atom tile t, 6144 int16 indices wrapped as
    # [16 partitions, 384] and replicated across the 8 partition groups
    IDXW = (128 * kk) // 16
    idx_d = nc.declare_dram_parameter("idx16", [128, NT * IDXW], I16, isOutput=False)
    out_d = nc.declare_dram_parameter("out", [A_PAD, NOUT], F32, isOutput=True)

    # --- per-core DRAM scratch: the full atom-embedding table ---
    y_d = nc.dram_tensor("y_table", [YROWS, NF], F32)

    dbg_y_d = dbg_yg_d = dbg_z_d = None
    if debug:
        dbg_y_d = nc.declare_dram_parameter("dbg_y", [YROWS, NF], F32, isOutput=True)
        dbg_yg_d = nc.declare_dram_parameter("dbg_yg", [128, kk * NF], F32, isOutput=True)
        dbg_z_d = nc.declare_dram_parameter("dbg_z", [A_PAD, NF], F32, isOutput=True)

    with tile.TileContext(nc) as tc, ExitStack() as ctx:
        const = ctx.enter_context(tc.tile_pool(name="const", bufs=1))
        psA = ctx.enter_context(tc.tile_pool(name="psA", bufs=2, space="PSUM"))
        psB = ctx.enter_context(tc.tile_pool(name="psB", bufs=4, space="PSUM"))
        psC = ctx.enter_context(tc.tile_pool(name="psC", bufs=2, space="PSUM"))
        sb_slab = ctx.enter_context(tc.tile_pool(name="slab", bufs=2))
        sb_h1 = ctx.enter_context(tc.tile_pool(name="h1", bufs=4))
        sb_yg = ctx.enter_context(tc.tile_pool(name="yg", bufs=3))
        sb_p = ctx.enter_context(tc.tile_pool(name="prod", bufs=3))
        sb_z = ctx.enter_context(tc.tile_pool(name="z", bufs=2))
        sb_f2 = ctx.enter_context(tc.tile_pool(name="f2", bufs=2))

        # ---- constants ----
        w1_sb = const.tile([NG, NF], F32)
        nc.sync.dma_start(w1_sb[:], w1_d[:, :])
        w2_sb = const.tile([NF, NF], F32)
        nc.sync.dma_start(w2_sb[:], w2_d[:, :])
        w_in2f_sb = const.tile([NIN, NF], F32)
        nc.sync.dma_start(w_in2f_sb[:], w_in2f_d[:, :])
        wf_sb = const.tile([NF, NOUT], F32)
        nc.sync.dma_start(wf_sb[:], wf_d[:, :])
        b1_sb = const.tile([NF, 1], F32)
        nc.sync.dma_start(b1_sb[:], b1_d[:, :])
        bf_sb = const.tile([NOUT, 1], F32)
        nc.sync.dma_start(bf_sb[:], bf_d[:, :])
        ident = const.tile([128, 128], F32)
        make_identity(nc, ident[:])
        half_sb = const.tile([128, 1], F32)
        nc.gpsimd.memset(half_sb[:], 0.5)
        idx_sb = const.tile([128, NT, IDXW], I16)
        nc.sync.dma_start(idx_sb[:], idx_d[:, :].rearrange("p (t w) -> p t w", t=NT))

        # ---- phase 1: build the y table (y = x @ W_in2f), store to DRAM ----
        with tc.tile_pool(name="xT", bufs=1) as sb_x, tc.tile_pool(
            name="ysb", bufs=2
        ) as sb_y:
            xT_sb = sb_x.tile([NIN, YROWS], F32)
            for xc in range(4):
                c0, c1 = (YROWS * xc) // 4, (YROWS * (xc + 1)) // 4
                nc.sync.dma_start(xT_sb[:, c0:c1], xT_d[:, c0:c1])
            BATCH = 4
            nb_done = 0
            for b in range((YT_TILES + BATCH - 1) // BATCH):
                nb = min(BATCH, YT_TILES - nb_done)
                y_sb = sb_y.tile([128, BATCH, NF], F32)
                for i in range(nb):
                    t = nb_done + i
                    y_ps = psA.tile([128, NF], F32, tag="mm1")
                    nc.tensor.matmul(
                        y_ps[:],
                        lhsT=xT_sb[:, t * 128 : (t + 1) * 128],
                        rhs=w_in2f_sb[:],
                        start=True,
                        stop=True,
                    )
                    nc.any.tensor_copy(y_sb[:, i, :], y_ps[:])
                nc.sync.dma_start(
                    y_d[nb_done * 128 : (nb_done + nb) * 128, :].rearrange(
                        "(t p) f -> p t f", p=128
                    ),
                    y_sb[:, :nb, :],
                )
                if debug:
                    nc.sync.dma_start(
                        dbg_y_d[nb_done * 128 : (nb_done + nb) * 128, :].rearrange(
                            "(t p) f -> p t f", p=128
                        ),
                        y_sb[:, :nb, :],
                    )
                nb_done += nb

        # view of dRexpT as [g, t, k, i] (t-major so each tile's slab is one
        # contiguous run per partition -> 25 DMA descriptors instead of 1200)
        dRexpT_v = dRexpT_d[:, :].rearrange(
            "g (t k i) -> g t k i", t=NT, k=kk, i=128
        )
        NQ = kk // 4  # quad-groups of neighbor slots

        # ---- phase 2: filter net + gather + weighted aggregation ----
        # ssp(v) = softplus(v) - log2 = ln(0.5*exp(v) + 0.5), built from the
        # exp+ln ACT table set (no softplus table exists on trn2).
        def issue_gather(t):
            yg = sb_yg.tile([128, kk, NF], F32, tag="yg", name=f"yg{t}")
            nc.gpsimd.dma_gather(
                out_ap=yg[:],
                in_ap=y_d[:, :],
                idxs_ap=idx_sb[:, t, :],
                num_idxs=128 * kk,
                num_idxs_reg=128 * kk,
                elem_size=NF,
                single_packet=False,
                queue_num=t % 4,
            )
            return yg

        # prefetch gathers two tiles ahead so descriptor generation on the
        # gpsimd engine runs back-to-back instead of trailing each tile's
        # compute loop
        pending = [issue_gather(0), issue_gather(1), issue_gather(2)]
        for t in range(NT):
            yg = pending[t]
            if t + 3 < NT:
                pending.append(issue_gather(t + 3))

            if debug and t == 0:
                nc.sync.dma_start(dbg_yg_d[:, :], yg[:].rearrange("p a b -> p (a b)"))

            zw = sb_z.tile([128, 512], F32, tag="zw")
            nc.vector.memset(zw[:], 0.0)

            # all 48 slots of dRexp^T for this atom tile
            slab = sb_slab.tile([NG, kk, 128], F32, tag="slab")
            nc.sync.dma_start(slab[:], dRexpT_v[:, t, :, :])

            for q in range(NQ):
                # mm1: h1^T [f, 512] over 4 neighbor slots x 128 atoms
                h1_ps = psA.tile([128, 512], F32, tag="mm1")
                # float32r: single-pass fp32 matmul mode, 4x faster streaming
                # at >=256 rows (vs 4 cycles/row for plain fp32)
                nc.tensor.matmul(
                    h1_ps[:],
                    lhsT=w1_sb[:],
                    rhs=slab[:, q * 4 : (q + 1) * 4, :].rearrange("g a b -> g (a b)"),
                    start=True,
                    stop=True,
                )
                u_sb = sb_h1.tile([128, 512], F32, tag="u")
                nc.scalar.activation(u_sb[:], h1_ps[:], ACTF.Exp, bias=b1_sb[:, :1])
                h1s = sb_h1.tile([128, 512], F32, tag="h1s")
                nc.scalar.activation(h1s[:], u_sb[:], ACTF.Ln, bias=half_sb[:, :1], scale=0.5)

                # mm2: W [e,h] per 128-edge tile, 4 tiles packed in one bank
                wq = psB.tile([128, 512], F32, tag="wq")
                for j in range(4):
                    nc.tensor.matmul(
                        wq[:, j * 128 : (j + 1) * 128],
                        lhsT=h1s[:, j * 128 : (j + 1) * 128],
                        rhs=w2_sb[:],
                        start=(j == 0),
                        stop=(j == 3),
                    )

                # weighted product with gathered neighbor embeddings,
                # accumulated 4-slots-wide; folded to 128 after the k loop
                p = sb_p.tile([128, 512], F32, tag="prod")
                nc.vector.tensor_tensor(
                    p[:],
                    wq[:],
                    yg[:, q * 4 : (q + 1) * 4, :].rearrange("p a b -> p (a b)"),
                    AOP.mult,
                )
                nc.vector.tensor_tensor(zw[:], zw[:], p[:], AOP.add)

            zh = sb_z.tile([128, 256], F32, tag="zh")
            nc.vector.tensor_tensor(zh[:], zw[:, 0:256], zw[:, 256:512], AOP.add)
            z = sb_z.tile([128, NF], F32, tag="z")
            nc.vector.tensor_tensor(z[:], zh[:, 0:128], zh[:, 128:256], AOP.add)

            if debug:
                nc.sync.dma_start(dbg_z_d[t * 128 : (t + 1) * 128, :], z[:])

            # ---- f2out ----
            zT_ps = psC.tile([128, 128], F32, tag="f2ps")
            nc.tensor.transpose(zT_ps[:], z[:], ident[:])
            zT_sb = sb_f2.tile([128, 128], F32, tag="zT")
            nc.vector.tensor_copy(zT_sb[:], zT_ps[:])
            o_ps = psC.tile([128, 128], F32, tag="f2ps")
            nc.tensor.matmul(
                o_ps[:], lhsT=wf_sb[:], rhs=zT_sb[:], start=True, stop=True
            )
            uo_sb = sb_f2.tile([128, 128], F32, tag="uo")
            nc.scalar.activation(uo_sb[:], o_ps[:], ACTF.Exp, bias=bf_sb[:, :1])
            oT_sb = sb_f2.tile([128, 128], F32, tag="oT")
            nc.scalar.activation(oT_sb[:], uo_sb[:], ACTF.Ln, bias=half_sb[:, :1], scale=0.5)
            o2_ps = psC.tile([128, 128], F32, tag="f2ps")
            nc.tensor.transpose(o2_ps[:], oT_sb[:], ident[:])
            out_sb = sb_f2.tile([128, 128], F32, tag="osb")
            nc.vector.tensor_copy(out_sb[:], o2_ps[:])
            nc.sync.dma_start(out_d[t * 128 : (t + 1) * 128, :], out_sb[:])

    # Both Exp and Ln live in the "natural_log_exp_and_others" ACT table set,
    # but the table chooser assigns each func its first-containing set, which
    # alternates two sets and inserts a ~1.3us table reload per activation
    # (~290us of pure reload).  Restrict Exp/Ln to the shared set (scoped
    # patch around compile; set ids are positional so the dict is not
    # reordered).
    orig_tables = bacc.get_activation_tables

    def _one_set_tables(arch):
        t = orig_tables(arch)
        keep = "natural_log_exp_and_others"
        assert keep in t and ACTF.Exp in t[keep] and ACTF.Ln in t[keep]
        for name, funcs in t.items():
            if name != keep:
                for f in (ACTF.Exp, ACTF.Ln, ACTF.Copy, ACTF.Identity):
                    funcs.discard(f)
        return t

    bacc.get_activation_tables = _one_set_tables
    try:
        nc.compile()
    finally:
        bacc.get_activation_tables = orig_tables
    return nc


_NC_CACHE = {}


def _get_nc(kk=K):
    if kk not in _NC_CACHE:
        _NC_CACHE[kk] = build_nc(kk=kk)
    return _NC_CACHE[kk]


def make_in_maps(x, dR, dR_expanded, pairwise_mask, neighbors_idx,
                 W1, b1, W2, b2, W_in2f, W_f2out, b_f2out):
    x = np.asarray(x, np.float32)
    dR = np.asarray(dR, np.float32)
    dR_expanded = np.asarray(dR_expanded, np.float32)
    pairwise_mask = np.asarray(pairwise_mask, np.float32)
    neighbors_idx = np.asarray(neighbors_idx, np.int32)

    # x^T padded with zero columns -> y table rows >= N_ATOMS are exactly zero
    xT = np.zeros((NIN, YROWS), np.float32)
    xT[:, :N_ATOMS] = x.T

    common = {
        "xT": xT,
        "w_in2f": np.asarray(W_in2f, np.float32),
        "w1": np.asarray(W1, np.float32),
        "w2": np.asarray(W2, np.float32),
        "wf": np.asarray(W_f2out, np.float32),
        "b1": np.asarray(b1, np.float32).reshape(NF, 1),
        "b2": np.asarray(b2, np.float32).reshape(1, NF),
        "bf": np.asarray(b_f2out, np.float32).reshape(NOUT, 1),
    }

    # Sort each atom's neighbor slots valid-first (cutoff+mask), permuting
    # dRexp identically, then truncate the slot axis to the max valid count
    # (rounded to a quad).  Invalid slots within kk still gather the zero row.
    validF = (dR <= R_CUTOFF) & (pairwise_mask != 0.0)
    kk = int(min(K, -(-int(validF.sum(1).max()) // 4) * 4))

    in_maps = []
    for m in range(NCORES):
        sl = slice(m * A_CORE, (m + 1) * A_CORE)
        v = validF[sl]
        perm = np.argsort(~v, axis=1, kind="stable")[:, :kk]
        v_s = np.take_along_axis(v, perm, 1)
        idx_s = np.take_along_axis(neighbors_idx[sl], perm, 1)
        dre_s = np.take_along_axis(dR_expanded[sl], perm[:, :, None], 1)
        padded = np.zeros((A_PAD, kk, NG), np.float32)
        padded[:A_CORE] = dre_s
        # [g, t, k, i] layout: per atom tile, one contiguous slab per partition
        dRe = padded.reshape(NT, 128, kk, NG).transpose(3, 0, 2, 1)
        idxm = np.full((A_PAD, kk), ZIDX, np.int16)
        idxm[:A_CORE] = np.where(v_s, idx_s, ZIDX).astype(np.int16)
        # wrap for dma_gather: tile t's j-th gathered row (j = k*128 + p)
        # has its index at [partition j%16, slot j//16], replicated x8
        IDXW = (128 * kk) // 16
        idx16 = np.empty((128, NT * IDXW), np.int16)
        for t in range(NT):
            flat_t = idxm[t * 128 : (t + 1) * 128, :].T.reshape(-1)  # j = k*128+p
            wrapped = flat_t.reshape(IDXW, 16).T  # [16, IDXW]
            idx16[:, t * IDXW : (t + 1) * IDXW] = np.tile(wrapped, (8, 1))
        in_maps.append(
            {
                **common,
                "dRexpT": np.ascontiguousarray(dRe.reshape(NG, kk * A_PAD)),
                "idx16": idx16,
            }
        )
    return in_maps, kk


def kernel(**inputs) -> np.ndarray:
    from concourse.bass_utils import run_bass_kernel_spmd

    _check_b2(inputs["b2"])
    in_maps, kk = make_in_maps(**inputs)
    nc = _get_nc(kk)
    res = run_bass_kernel_spmd(nc, in_maps, list(range(NCORES)))
    outs = [np.asarray(res.results[m]["out"])[:A_CORE] for m in range(NCORES)]
    return np.concatenate(outs, axis=0)


# b2 handling note: reference adds b2 after the second filter matmul.  In this
# problem b2 == 0; the general case would fold b2 into the gather-product
# stage.  We assert on the host so a non-zero b2 cannot silently give wrong
# results.
def _check_b2(b2):
    assert np.all(np.asarray(b2) == 0.0), "kernel assumes b2 == 0"



# revision 7
# speedup vs baseline: 2.7769x; 2.7769x over previous
"""CFConv (SchNet continuous-filter convolution) on 8 Trainium2 NeuronCores.

Reference computation (per atom i, neighbor slot k):
    W[i,k,:]  = ssp(dRexp[i,k,:] @ W1 + b1) @ W2 + b2       (filter network)
    C[i,k]    = (dR[i,k] <= 5.0)                            (hard cutoff)
    y         = x @ W_in2f                                  (atom embeddings)
    out[i,:]  = ssp( sum_k C*mask*W[i,k,:]*y[nbh[i,k],:] @ W_f2out + b_f2out )
    where ssp(v) = softplus(v) - log(2) = ln(0.5*e^v + 0.5)

Sharding: atoms split across 8 cores (1250 each).

Design decisions (vs. the original on-device-gather kernel):

1. HOST PRE-GATHER.  y = x @ W_in2f is one cheap host matmul; the per-edge
   expansion y[nbh[i,k],:] is a numpy take.  The device receives a dense
   pre-gathered feature-major fp32 tensor ygT[f, e] per core and performs no
   data-dependent access.  (The on-device dma_gather burned ~370us/core of
   serialized gpsimd descriptor generation plus 23.6MB of 512B random HBM
   packets -- the dominant cost of the old kernel.)

2. ALL-FP32 DATA PATH.  The harness metric is max |err|/max(|expected|,1e-2);
   near-zero outputs need ~2e-4 ABSOLUTE accuracy.  Empirically every 16-bit
   (or tf32=float32r) stage alone measures 0.1-0.3 on this metric, so inputs,
   matmuls (plain fp32, 4 cyc/col), products and sums all stay fp32.

3. VALID-COUNT BUCKETING.  Atoms are host-sorted by their valid-neighbor
   count rounded up to a multiple of 4; each bucket kk processes only kk
   neighbor slots per atom instead of a uniform worst-case 36, cutting edge
   volume (and with it every engine's work + DMA bytes) by ~27%.  All 8 cores
   run one SPMD program, so bucket capacities are the max over cores; pad
   chunks are all-zero and their outputs are ignored by the host unpack.

Device pipeline per bucket (feature-major: filters on partitions, edges on
the free dim, edge order e = atom-major (a, k), chunks padded to 512 columns
so each chunk fills exactly one fp32 PSUM bank):
  mm1:  h1T[f, e]  = W1^T @ dRexpT[g, e]       (W1 stationary)
  ACT:  u = exp(h1T + b1)                      (PSUM -> SBUF, per 1024 cols)
  ACT:  h1sT = ln(0.5*u + 0.5) = ssp(h1T)      (exact, shared exp/ln table)
  mm2:  WT[h, e]   = W2^T @ h1sT               (W2 stationary)
  DVE:  prodT[h,e] = WT_psum * ygT             (masked/padded edges have yg=0)
  DVE:  zT[h, a]   = k-slot sum (one strided tensor_reduce per slab)
  f2out: outT[o,:] = ssp(Wf^T @ zT + bf)       (matmul + exp + ln)
Host transposes outT back to [atoms, features] and undoes the bucket sort.

Both Exp and Ln live in the "natural_log_exp_and_others" ACT table set; the
table chooser is patched so no per-instruction activation-table reloads
happen.
"""

import numpy as np
from contextlib import ExitStack

import concourse.bass as bass
import concourse.bacc as bacc
import concourse.mybir as mybir
import concourse.tile as tile

F32 = mybir.dt.float32
AOP = mybir.AluOpType
ACTF = mybir.ActivationFunctionType
AXIS = mybir.AxisListType

# ---- geometry (hardcoded for nn_CFConv_13245679141058) ----
N_ATOMS = 10000
K = 48                    # neighbors per atom
NIN = NF = NOUT = 128
NG = 25                   # gaussians
NCORES = 8
A_CORE = N_ATOMS // NCORES        # 1250 real atoms per core
CH = 512                          # columns per chunk (= one fp32 PSUM bank)
SLAB_CH = 6                       # chunks per DMA slab
R_CUTOFF = 5.0
LOG2 = float(np.log(2.0))


def build_nc(sig):
    """sig: tuple of (kk, nch) per bucket; nch even, chunk = 512 cols holding
    a_ch = 512//kk atoms of kk slots each (tail cols zero)."""
    ecols = sum(nch * CH for _, nch in sig)
    a_pad = sum(nch * (CH // kk) for kk, nch in sig)

    nc = bacc.Bacc()

    ygT_d = nc.declare_dram_parameter("ygT", [NF, ecols], F32, isOutput=False)
    dreT_d = nc.declare_dram_parameter("dreT", [NG, ecols], F32, isOutput=False)
    w1_d = nc.declare_dram_parameter("w1", [NG, NF], F32, isOutput=False)
    w2_d = nc.declare_dram_parameter("w2", [NF, NF], F32, isOutput=False)
    wf_d = nc.declare_dram_parameter("wf", [NF, NOUT], F32, isOutput=False)
    b1_d = nc.declare_dram_parameter("b1", [NF, 1], F32, isOutput=False)
    bf_d = nc.declare_dram_parameter("bf", [NOUT, 1], F32, isOutput=False)
    outT_d = nc.declare_dram_parameter("outT", [NOUT, a_pad], F32, isOutput=True)

    with tile.TileContext(nc) as tc, ExitStack() as ctx:
        const = ctx.enter_context(tc.tile_pool(name="const", bufs=1))
        sb_yg = ctx.enter_context(tc.tile_pool(name="yg", bufs=3))
        sb_dre = ctx.enter_context(tc.tile_pool(name="dre", bufs=3))
        sb_u = ctx.enter_context(tc.tile_pool(name="u", bufs=2))
        sb_h = ctx.enter_context(tc.tile_pool(name="h1s", bufs=2))
        sb_p = ctx.enter_context(tc.tile_pool(name="prod", bufs=2))
        sb_z = ctx.enter_context(tc.tile_pool(name="z", bufs=1))
        sb_o = ctx.enter_context(tc.tile_pool(name="f2o", bufs=2))
        psH = ctx.enter_context(tc.tile_pool(name="psH", bufs=2, space="PSUM"))
        psW = ctx.enter_context(tc.tile_pool(name="psW", bufs=3, space="PSUM"))
        psO = ctx.enter_context(tc.tile_pool(name="psO", bufs=1, space="PSUM"))

        # ---- constants ----
        w1_sb = const.tile([NG, NF], F32)
        nc.sync.dma_start(w1_sb[:], w1_d[:, :])
        w2_sb = const.tile([NF, NF], F32)
        nc.sync.dma_start(w2_sb[:], w2_d[:, :])
        wf_sb = const.tile([NF, NOUT], F32)
        nc.sync.dma_start(wf_sb[:], wf_d[:, :])
        b1_sb = const.tile([NF, 1], F32)
        nc.sync.dma_start(b1_sb[:], b1_d[:, :])
        bf_sb = const.tile([NOUT, 1], F32)
        nc.sync.dma_start(bf_sb[:], bf_d[:, :])
        half_sb = const.tile([128, 1], F32)
        nc.vector.memset(half_sb[:], 0.5)

        zT = sb_z.tile([NF, a_pad], F32)
        SC = SLAB_CH * CH
        ecoff = 0                     # column offset into ygT/dreT
        zoff = 0                      # column offset into zT

        for kk, nch in sig:
            a_ch = CH // kk
            for s0 in range(0, nch, SLAB_CH):
                sc = min(SLAB_CH, nch - s0)   # chunks in this slab (even)
                yg_sl = sb_yg.tile([NF, SC], F32, tag="yg")
                nc.sync.dma_start(
                    yg_sl[:, : sc * CH], ygT_d[:, ecoff : ecoff + sc * CH]
                )
                dre_sl = sb_dre.tile([NG, SC], F32, tag="dre")
                nc.sync.dma_start(
                    dre_sl[:, : sc * CH], dreT_d[:, ecoff : ecoff + sc * CH]
                )
                u_sl = sb_u.tile([NF, SC], F32, tag="u")
                h_sl = sb_h.tile([NF, SC], F32, tag="h1s")
                p_sl = sb_p.tile([NF, SC], F32, tag="prod")

                for h in range(sc // 2):      # chunk pairs
                    h1_ps = psH.tile([NF, 2 * CH], F32, tag="mm1")
                    for j in range(2):
                        c = 2 * h + j
                        nc.tensor.matmul(
                            h1_ps[:, j * CH : (j + 1) * CH],
                            lhsT=w1_sb[:],
                            rhs=dre_sl[:, c * CH : (c + 1) * CH],
                            start=True,
                            stop=True,
                        )
                    # ssp(v) = ln(0.5*e^v + 0.5), shared exp/ln table set
                    nc.scalar.activation(
                        u_sl[:, h * 2 * CH : (h + 1) * 2 * CH],
                        h1_ps[:],
                        ACTF.Exp,
                        bias=b1_sb[:, :1],
                    )
                    nc.scalar.activation(
                        h_sl[:, h * 2 * CH : (h + 1) * 2 * CH],
                        u_sl[:, h * 2 * CH : (h + 1) * 2 * CH],
                        ACTF.Ln,
                        bias=half_sb[:, :1],
                        scale=0.5,
                    )
                    for j in range(2):
                        c = 2 * h + j
                        w_ps = psW.tile([NF, CH], F32, tag="mm2")
                        nc.tensor.matmul(
                            w_ps[:],
                            lhsT=w2_sb[:],
                            rhs=h_sl[:, c * CH : (c + 1) * CH],
                            start=True,
                            stop=True,
                        )
                        nc.vector.tensor_tensor(
                            p_sl[:, c * CH : (c + 1) * CH],
                            w_ps[:],
                            yg_sl[:, c * CH : (c + 1) * CH],
                            AOP.mult,
                        )

                # one strided k-sum over the slab: p_sl viewed as
                # [f, chunk, atom, k]; 512-col pad tails skipped by the AP
                nc.vector.tensor_reduce(
                    zT[:, zoff : zoff + sc * a_ch],
                    p_sl[:, : sc * CH]
                    .rearrange("f (c x) -> f c x", c=sc)[:, :, : a_ch * kk]
                    .rearrange("f c (a k) -> f c a k", k=kk),
                    axis=AXIS.X,
                    op=AOP.add,
                )
                ecoff += sc * CH
                zoff += sc * a_ch

        # ---- f2out: outT = ssp(Wf^T @ zT + bf) ----
        for off in range(0, a_pad, CH):
            n = min(CH, a_pad - off)
            o_ps = psO.tile([NOUT, n], F32, tag="f2")
            nc.tensor.matmul(
                o_ps[:], lhsT=wf_sb[:], rhs=zT[:, off : off + n],
                start=True, stop=True,
            )
            u2 = sb_o.tile([NOUT, n], F32, tag="u2")
            nc.scalar.activation(u2[:], o_ps[:], ACTF.Exp, bias=bf_sb[:, :1])
            o_sb = sb_o.tile([NOUT, n], F32, tag="o")
            nc.scalar.activation(
                o_sb[:], u2[:], ACTF.Ln, bias=half_sb[:, :1], scale=0.5
            )
            nc.sync.dma_start(outT_d[:, off : off + n], o_sb[:])

    # Pin Exp/Ln (and Copy/Identity) to the single shared table set so the
    # table chooser never inserts mid-kernel activation-table reloads.
    orig_tables = bacc.get_activation_tables

    def _one_set_tables(arch):
        t = orig_tables(arch)
        keep = "natural_log_exp_and_others"
        assert keep in t and ACTF.Exp in t[keep] and ACTF.Ln in t[keep]
        for name, funcs in t.items():
            if name != keep:
                for f in (ACTF.Exp, ACTF.Ln, ACTF.Copy, ACTF.Identity):
                    funcs.discard(f)
        return t

    bacc.get_activation_tables = _one_set_tables
    try:
        nc.compile()
    finally:
        bacc.get_activation_tables = orig_tables
    return nc


_NC_CACHE = {}


def _get_nc(sig):
    if sig not in _NC_CACHE:
        _NC_CACHE[sig] = build_nc(sig)
    return _NC_CACHE[sig]


def _make_sig(validF):
    """Shared SPMD bucket signature: (kk, nch) per bucket, kk ascending.
    nch = max chunk count over cores, rounded up to even.  Levels whose
    largest per-core population fills < 2 chunks are merged upward; a
    trailing sparse level bumps the last bucket's kk instead (so no atom
    ever lands in a bucket smaller than its valid count)."""
    v = validF.sum(1).astype(np.int64).reshape(NCORES, A_CORE)
    kk4 = np.clip(((v + 3) // 4) * 4, 4, K)
    levels = sorted(set(kk4.ravel().tolist()))
    counts = {kk: (kk4 == kk).sum(axis=1) for kk in levels}
    buckets = []                       # [kk, per-core n]
    carry = np.zeros(NCORES, np.int64)
    for i, kk in enumerate(levels):
        n = counts[kk] + carry
        if i + 1 < len(levels) and n.max() < 2 * (CH // kk):
            carry = n
            continue
        buckets.append([kk, n])
        carry = np.zeros(NCORES, np.int64)
    if carry.max() > 0:
        if buckets:
            buckets[-1][0] = levels[-1]
            buckets[-1][1] = buckets[-1][1] + carry
        else:
            buckets = [[levels[-1], carry]]
    sig = []
    for kk, n in buckets:
        a_ch = CH // kk
        nch = int(-(-int(n.max()) // a_ch))
        nch += nch % 2
        sig.append((kk, nch))
    return tuple(sig), kk4


def make_in_maps(x, dR, dR_expanded, pairwise_mask, neighbors_idx,
                 W1, b1, W2, b2, W_in2f, W_f2out, b_f2out):
    x = np.asarray(x, np.float32)
    dR = np.asarray(dR, np.float32)
    dR_expanded = np.asarray(dR_expanded, np.float32)
    pairwise_mask = np.asarray(pairwise_mask, np.float32)
    neighbors_idx = np.asarray(neighbors_idx, np.int64)

    # atom embeddings + cutoff/mask folded into the host-side gather
    y = x @ np.asarray(W_in2f, np.float32)                  # [N, F]
    validF = (dR <= R_CUTOFF) & (pairwise_mask != 0.0)
    sig, kk4 = _make_sig(validF)
    sig_kks = np.array([kk for kk, _ in sig], np.int64)
    ecols = sum(nch * CH for _, nch in sig)
    a_pad = sum(nch * (CH // kk) for kk, nch in sig)

    common = {
        "w1": np.asarray(W1, np.float32),
        "w2": np.asarray(W2, np.float32),
        "wf": np.asarray(W_f2out, np.float32),
        "b1": np.asarray(b1, np.float32).reshape(NF, 1),
        "bf": np.asarray(b_f2out, np.float32).reshape(NOUT, 1),
    }

    in_maps = []
    slots = []                       # per core: z-column slot of each atom
    for m in range(NCORES):
        sl = slice(m * A_CORE, (m + 1) * A_CORE)
        v = validF[sl]
        # each atom goes to the first bucket with kk >= its rounded count
        abkt = np.searchsorted(sig_kks, kk4[m])
        order = np.argsort(abkt, kind="stable")

        ygT = np.zeros((NF, ecols), np.float32)
        dreT = np.zeros((NG, ecols), np.float32)
        slot = np.zeros(A_CORE, np.int64)

        ecoff = 0
        zoff = 0
        for bi, (kk, nch) in enumerate(sig):
            a_ch = CH // kk
            atoms = order[abkt[order] == bi]
            n = len(atoms)
            vb = v[atoms][:, :]                       # [n, K] valid masks
            perm = np.argsort(~vb, axis=1, kind="stable")[:, :kk]
            v_s = np.take_along_axis(vb, perm, 1)
            idx_s = np.take_along_axis(neighbors_idx[sl][atoms], perm, 1)
            dre_s = np.take_along_axis(
                dR_expanded[sl][atoms], perm[:, :, None], 1
            )
            n_pad = nch * a_ch
            yg = np.zeros((n_pad, kk, NF), np.float32)
            yg[:n] = np.where(v_s[..., None], y[idx_s], 0.0)
            dre = np.zeros((n_pad, kk, NG), np.float32)
            dre[:n] = dre_s
            # chunk layout: [nch, 512] cols; first a_ch*kk real, tail zero
            blk = ygT[:, ecoff : ecoff + nch * CH].reshape(NF, nch, CH)
            blk[:, :, : a_ch * kk] = (
                yg.reshape(nch, a_ch * kk, NF).transpose(2, 0, 1)
            )
            blk = dreT[:, ecoff : ecoff + nch * CH].reshape(NG, nch, CH)
            blk[:, :, : a_ch * kk] = (
                dre.reshape(nch, a_ch * kk, NG).transpose(2, 0, 1)
            )
            slot[atoms] = zoff + np.arange(n)
            ecoff += nch * CH
            zoff += n_pad

        slots.append(slot)
        in_maps.append({**common, "ygT": ygT, "dreT": dreT})
    return in_maps, sig, slots


def kernel(**inputs) -> np.ndarray:
    from concourse.bass_utils import run_bass_kernel_spmd

    _check_b2(inputs["b2"])
    in_maps, sig, slots = make_in_maps(**inputs)
    nc = _get_nc(sig)
    res = run_bass_kernel_spmd(nc, in_maps, list(range(NCORES)))
    outs = []
    for m in range(NCORES):
        outT = np.asarray(res.results[m]["outT"])       # [NOUT, a_pad]
        outs.append(np.ascontiguousarray(outT.T[slots[m]]))
    return np.concatenate(outs, axis=0)


# b2 handling note: reference adds b2 after the second filter matmul.  In this
# problem b2 == 0; the general case would fold b2 into the product stage (the
# extra term is b2[h] * sum_k yg[i,k,h], computable host-side).  Assert so a
# non-zero b2 cannot silently give wrong results.
def _check_b2(b2):
    assert np.all(np.asarray(b2) == 0.0), "kernel assumes b2 == 0"


# revision 10
# speedup vs baseline: 3.4088x; 1.2276x over previous
"""CFConv (SchNet continuous-filter convolution) on 8 Trainium2 NeuronCores.

Reference computation (per atom i, neighbor slot k):
    W[i,k,:]  = ssp(dRexp[i,k,:] @ W1 + b1) @ W2 + b2       (filter network)
    C[i,k]    = (dR[i,k] <= 5.0)                            (hard cutoff)
    y         = x @ W_in2f                                  (atom embeddings)
    out[i,:]  = ssp( sum_k C*mask*W[i,k,:]*y[nbh[i,k],:] @ W_f2out + b_f2out )
    where ssp(v) = softplus(v) - log(2) = ln(0.5*e^v + 0.5)

Sharding: atoms split across 8 cores (1250 each).

Design decisions (vs. the original on-device-gather kernel):

1. HOST PRE-GATHER.  y = x @ W_in2f is one cheap host matmul; the per-edge
   expansion y[nbh[i,k],:] is a numpy take.  The device receives a dense
   pre-gathered feature-major fp32 tensor ygT[f, e] per core and performs no
   data-dependent access.  (The on-device dma_gather burned ~370us/core of
   serialized gpsimd descriptor generation plus 23.6MB of 512B random HBM
   packets -- the dominant cost of the old kernel.)

2. ALL-FP32 DATA PATH.  The harness metric is max |err|/max(|expected|,1e-2);
   near-zero outputs need ~2e-4 ABSOLUTE accuracy.  Empirically every 16-bit
   (or tf32=float32r) stage alone measures 0.1-0.3 on this metric, so inputs,
   matmuls (plain fp32, 4 cyc/col), products and sums all stay fp32.

3. VALID-COUNT BUCKETING.  Atoms are host-sorted by their valid-neighbor
   count rounded up to a multiple of 4; each bucket kk processes only kk
   neighbor slots per atom instead of a uniform worst-case 36, cutting edge
   volume (and with it every engine's work + DMA bytes) by ~27%.  All 8 cores
   run one SPMD program, so bucket capacities are the max over cores; pad
   chunks are all-zero and their outputs are ignored by the host unpack.

Device pipeline per bucket (feature-major: filters on partitions, edges on
the free dim, edge order e = atom-major (a, k), chunks padded to 512 columns
so each chunk fills exactly one fp32 PSUM bank):
  mm1:  h1T[f, e]  = W1^T @ dRexpT[g, e]       (W1 stationary)
  ACT:  u = exp(h1T + b1)                      (PSUM -> SBUF, per 1024 cols)
  ACT:  h1sT = ln(0.5*u + 0.5) = ssp(h1T)      (exact, shared exp/ln table)
  mm2:  WT[h, e]   = W2^T @ h1sT               (W2 stationary)
  DVE:  prodT[h,e] = WT_psum * ygT             (masked/padded edges have yg=0)
  DVE:  zT[h, a]   = k-slot sum (one strided tensor_reduce per slab)
  f2out: outT[o,:] = ssp(Wf^T @ zT + bf)       (matmul + exp + ln)
Host transposes outT back to [atoms, features] and undoes the bucket sort.

Both Exp and Ln live in the "natural_log_exp_and_others" ACT table set; the
table chooser is patched so no per-instruction activation-table reloads
happen.
"""

import numpy as np
from contextlib import ExitStack

import concourse.bass as bass
import concourse.bacc as bacc
import concourse.mybir as mybir
import concourse.tile as tile

F32 = mybir.dt.float32
AOP = mybir.AluOpType
ACTF = mybir.ActivationFunctionType
AXIS = mybir.AxisListType

# ---- geometry (hardcoded for nn_CFConv_13245679141058) ----
N_ATOMS = 10000
K = 48                    # neighbors per atom
NIN = NF = NOUT = 128
NG = 25                   # gaussians
NCORES = 8
A_CORE = N_ATOMS // NCORES        # 1250 real atoms per core
CH = 512                          # columns per chunk (= one fp32 PSUM bank)
SLAB_CH = 4                       # chunks per DMA slab
R_CUTOFF = 5.0
LOG2 = float(np.log(2.0))


def build_nc(sig):
    """sig: tuple of (kk, nch) per bucket; nch even, chunk = 512 cols holding
    a_ch = 512//kk atoms of kk slots each (tail cols zero)."""
    ecols = sum(nch * CH for _, nch in sig)
    a_pad = sum(nch * (CH // kk) for kk, nch in sig)

    nc = bacc.Bacc()

    ygT_d = nc.declare_dram_parameter("ygT", [NF, ecols], F32, isOutput=False)
    dreT_d = nc.declare_dram_parameter("dreT", [NG, ecols], F32, isOutput=False)
    w1_d = nc.declare_dram_parameter("w1", [NG, NF], F32, isOutput=False)
    w2_d = nc.declare_dram_parameter("w2", [NF, NF], F32, isOutput=False)
    wf_d = nc.declare_dram_parameter("wf", [NF, NOUT], F32, isOutput=False)
    b1_d = nc.declare_dram_parameter("b1", [NF, 1], F32, isOutput=False)
    bf_d = nc.declare_dram_parameter("bf", [NOUT, 1], F32, isOutput=False)
    outT_d = nc.declare_dram_parameter("outT", [NOUT, a_pad], F32, isOutput=True)

    with tile.TileContext(nc) as tc, ExitStack() as ctx:
        const = ctx.enter_context(tc.tile_pool(name="const", bufs=1))
        sb_yg = ctx.enter_context(tc.tile_pool(name="yg", bufs=4))
        sb_dre = ctx.enter_context(tc.tile_pool(name="dre", bufs=4))
        sb_u = ctx.enter_context(tc.tile_pool(name="u", bufs=3))
        sb_h = ctx.enter_context(tc.tile_pool(name="h1s", bufs=3))
        sb_p = ctx.enter_context(tc.tile_pool(name="prod", bufs=3))
        sb_z = ctx.enter_context(tc.tile_pool(name="z", bufs=1))
        sb_o = ctx.enter_context(tc.tile_pool(name="f2o", bufs=2))
        psH = ctx.enter_context(tc.tile_pool(name="psH", bufs=2, space="PSUM"))
        psW = ctx.enter_context(tc.tile_pool(name="psW", bufs=3, space="PSUM"))
        psO = ctx.enter_context(tc.tile_pool(name="psO", bufs=1, space="PSUM"))

        # ---- constants ----
        w1_sb = const.tile([NG, NF], F32)
        nc.sync.dma_start(w1_sb[:], w1_d[:, :])
        w2_sb = const.tile([NF, NF], F32)
        nc.sync.dma_start(w2_sb[:], w2_d[:, :])
        wf_sb = const.tile([NF, NOUT], F32)
        nc.sync.dma_start(wf_sb[:], wf_d[:, :])
        b1_sb = const.tile([NF, 1], F32)
        nc.sync.dma_start(b1_sb[:], b1_d[:, :])
        bf_sb = const.tile([NOUT, 1], F32)
        nc.sync.dma_start(bf_sb[:], bf_d[:, :])
        half_sb = const.tile([128, 1], F32)
        nc.vector.memset(half_sb[:], 0.5)

        zT = sb_z.tile([NF, a_pad], F32)
        SC = SLAB_CH * CH

        # flatten the (bucket, slab, pair) structure so emission can be
        # software-pipelined across slab/bucket boundaries
        slabs = []
        ecoff = zoff = 0
        for kk, nch in sig:
            a_ch = CH // kk
            for s0 in range(0, nch, SLAB_CH):
                sc = min(SLAB_CH, nch - s0)   # chunks in this slab (even)
                slabs.append(
                    {"kk": kk, "a_ch": a_ch, "sc": sc,
                     "ecoff": ecoff, "zoff": zoff, "tiles": None}
                )
                ecoff += sc * CH
                zoff += sc * a_ch
        pairs = [(si, h) for si, sl in enumerate(slabs)
                 for h in range(sl["sc"] // 2)]

        def tiles(si):
            sl = slabs[si]
            if sl["tiles"] is None:
                sc = sl["sc"]
                yg_sl = sb_yg.tile([NF, SC], F32, tag="yg")
                nc.sync.dma_start(
                    yg_sl[:, : sc * CH],
                    ygT_d[:, sl["ecoff"] : sl["ecoff"] + sc * CH],
                )
                dre_sl = sb_dre.tile([NG, SC], F32, tag="dre")
                nc.sync.dma_start(
                    dre_sl[:, : sc * CH],
                    dreT_d[:, sl["ecoff"] : sl["ecoff"] + sc * CH],
                )
                u_sl = sb_u.tile([NF, SC], F32, tag="u")
                h_sl = sb_h.tile([NF, SC], F32, tag="h1s")
                p_sl = sb_p.tile([NF, SC], F32, tag="prod")
                sl["tiles"] = (yg_sl, dre_sl, u_sl, h_sl, p_sl)
            return sl["tiles"]

        def front(i):
            """mm1 pair + exp + ln for pairs[i]."""
            si, h = pairs[i]
            _, dre_sl, u_sl, h_sl, _ = tiles(si)
            h1_ps = psH.tile([NF, 2 * CH], F32, tag="mm1")
            for j in range(2):
                c = 2 * h + j
                nc.tensor.matmul(
                    h1_ps[:, j * CH : (j + 1) * CH],
                    lhsT=w1_sb[:],
                    rhs=dre_sl[:, c * CH : (c + 1) * CH],
                    start=True,
                    stop=True,
                )
            # ssp(v) = ln(0.5*e^v + 0.5), shared exp/ln table set
            nc.scalar.activation(
                u_sl[:, h * 2 * CH : (h + 1) * 2 * CH],
                h1_ps[:],
                ACTF.Exp,
                bias=b1_sb[:, :1],
            )
            nc.scalar.activation(
                h_sl[:, h * 2 * CH : (h + 1) * 2 * CH],
                u_sl[:, h * 2 * CH : (h + 1) * 2 * CH],
                ACTF.Ln,
                bias=half_sb[:, :1],
                scale=0.5,
            )

        def back(i):
            """mm2 pair + products for pairs[i]; slab k-sum after its last
            pair."""
            si, h = pairs[i]
            sl = slabs[si]
            yg_sl, _, _, h_sl, p_sl = tiles(si)
            for j in range(2):
                c = 2 * h + j
                w_ps = psW.tile([NF, CH], F32, tag="mm2")
                nc.tensor.matmul(
                    w_ps[:],
                    lhsT=w2_sb[:],
                    rhs=h_sl[:, c * CH : (c + 1) * CH],
                    start=True,
                    stop=True,
                )
                nc.vector.tensor_tensor(
                    p_sl[:, c * CH : (c + 1) * CH],
                    w_ps[:],
                    yg_sl[:, c * CH : (c + 1) * CH],
                    AOP.mult,
                )
            if h == sl["sc"] // 2 - 1:
                sc, a_ch, kk = sl["sc"], sl["a_ch"], sl["kk"]
                # strided k-sum: p_sl as [f, chunk, atom, k]; pad tails
                # skipped by the AP
                nc.vector.tensor_reduce(
                    zT[:, sl["zoff"] : sl["zoff"] + sc * a_ch],
                    p_sl[:, : sc * CH]
                    .rearrange("f (c x) -> f c x", c=sc)[:, :, : a_ch * kk]
                    .rearrange("f c (a k) -> f c a k", k=kk),
                    axis=AXIS.X,
                    op=AOP.add,
                )

        # software-pipelined emission: the PE sees mm1(i+1) before mm2(i),
        # so it never idles while the ACT exp/ln chain for pair i finishes
        front(0)
        for i in range(len(pairs)):
            if i + 1 < len(pairs):
                front(i + 1)
            back(i)

        # ---- f2out: outT = ssp(Wf^T @ zT + bf) ----
        for off in range(0, a_pad, CH):
            n = min(CH, a_pad - off)
            o_ps = psO.tile([NOUT, n], F32, tag="f2")
            nc.tensor.matmul(
                o_ps[:], lhsT=wf_sb[:], rhs=zT[:, off : off + n],
                start=True, stop=True,
            )
            u2 = sb_o.tile([NOUT, n], F32, tag="u2")
            nc.scalar.activation(u2[:], o_ps[:], ACTF.Exp, bias=bf_sb[:, :1])
            o_sb = sb_o.tile([NOUT, n], F32, tag="o")
            nc.scalar.activation(
                o_sb[:], u2[:], ACTF.Ln, bias=half_sb[:, :1], scale=0.5
            )
            nc.sync.dma_start(outT_d[:, off : off + n], o_sb[:])

    # Pin Exp/Ln (and Copy/Identity) to the single shared table set so the
    # table chooser never inserts mid-kernel activation-table reloads.
    orig_tables = bacc.get_activation_tables

    def _one_set_tables(arch):
        t = orig_tables(arch)
        keep = "natural_log_exp_and_others"
        assert keep in t and ACTF.Exp in t[keep] and ACTF.Ln in t[keep]
        for name, funcs in t.items():
            if name != keep:
                for f in (ACTF.Exp, ACTF.Ln, ACTF.Copy, ACTF.Identity):
                    funcs.discard(f)
        return t

    bacc.get_activation_tables = _one_set_tables
    try:
        nc.compile()
    finally:
        bacc.get_activation_tables = orig_tables
    return nc


_NC_CACHE = {}


def _get_nc(sig):
    if sig not in _NC_CACHE:
        _NC_CACHE[sig] = build_nc(sig)
    return _NC_CACHE[sig]


def _make_sig(validF):
    """Shared SPMD bucket signature: (kk, nch) per bucket, kk ascending.
    nch = max chunk count over cores, rounded up to even.  Levels whose
    largest per-core population fills < 2 chunks are merged upward; a
    trailing sparse level bumps the last bucket's kk instead (so no atom
    ever lands in a bucket smaller than its valid count)."""
    v = validF.sum(1).astype(np.int64).reshape(NCORES, A_CORE)
    kk4 = np.clip(((v + 3) // 4) * 4, 4, K)
    levels = sorted(set(kk4.ravel().tolist()))
    counts = {kk: (kk4 == kk).sum(axis=1) for kk in levels}
    buckets = []                       # [kk, per-core n]
    carry = np.zeros(NCORES, np.int64)
    for i, kk in enumerate(levels):
        n = counts[kk] + carry
        if i + 1 < len(levels) and n.max() < 2 * (CH // kk):
            carry = n
            continue
        buckets.append([kk, n])
        carry = np.zeros(NCORES, np.int64)
    if carry.max() > 0:
        if buckets:
            buckets[-1][0] = levels[-1]
            buckets[-1][1] = buckets[-1][1] + carry
        else:
            buckets = [[levels[-1], carry]]
    sig = []
    for kk, n in buckets:
        a_ch = CH // kk
        nch = int(-(-int(n.max()) // a_ch))
        nch += nch % 2
        sig.append((kk, nch))
    return tuple(sig), kk4


def make_in_maps(x, dR, dR_expanded, pairwise_mask, neighbors_idx,
                 W1, b1, W2, b2, W_in2f, W_f2out, b_f2out):
    x = np.asarray(x, np.float32)
    dR = np.asarray(dR, np.float32)
    dR_expanded = np.asarray(dR_expanded, np.float32)
    pairwise_mask = np.asarray(pairwise_mask, np.float32)
    neighbors_idx = np.asarray(neighbors_idx, np.int64)

    # atom embeddings + cutoff/mask folded into the host-side gather
    y = x @ np.asarray(W_in2f, np.float32)                  # [N, F]
    validF = (dR <= R_CUTOFF) & (pairwise_mask != 0.0)
    sig, kk4 = _make_sig(validF)
    sig_kks = np.array([kk for kk, _ in sig], np.int64)
    ecols = sum(nch * CH for _, nch in sig)
    a_pad = sum(nch * (CH // kk) for kk, nch in sig)

    common = {
        "w1": np.asarray(W1, np.float32),
        "w2": np.asarray(W2, np.float32),
        "wf": np.asarray(W_f2out, np.float32),
        "b1": np.asarray(b1, np.float32).reshape(NF, 1),
        "bf": np.asarray(b_f2out, np.float32).reshape(NOUT, 1),
    }

    in_maps = []
    slots = []                       # per core: z-column slot of each atom
    for m in range(NCORES):
        sl = slice(m * A_CORE, (m + 1) * A_CORE)
        v = validF[sl]
        # each atom goes to the first bucket with kk >= its rounded count
        abkt = np.searchsorted(sig_kks, kk4[m])
        order = np.argsort(abkt, kind="stable")

        ygT = np.zeros((NF, ecols), np.float32)
        dreT = np.zeros((NG, ecols), np.float32)
        slot = np.zeros(A_CORE, np.int64)

        ecoff = 0
        zoff = 0
        for bi, (kk, nch) in enumerate(sig):
            a_ch = CH // kk
            atoms = order[abkt[order] == bi]
            n = len(atoms)
            vb = v[atoms][:, :]                       # [n, K] valid masks
            perm = np.argsort(~vb, axis=1, kind="stable")[:, :kk]
            v_s = np.take_along_axis(vb, perm, 1)
            idx_s = np.take_along_axis(neighbors_idx[sl][atoms], perm, 1)
            dre_s = np.take_along_axis(
                dR_expanded[sl][atoms], perm[:, :, None], 1
            )
            n_pad = nch * a_ch
            yg = np.zeros((n_pad, kk, NF), np.float32)
            yg[:n] = np.where(v_s[..., None], y[idx_s], 0.0)
            dre = np.zeros((n_pad, kk, NG), np.float32)
            dre[:n] = dre_s
            # chunk layout: [nch, 512] cols; first a_ch*kk real, tail zero
            blk = ygT[:, ecoff : ecoff + nch * CH].reshape(NF, nch, CH)
            blk[:, :, : a_ch * kk] = (
                yg.reshape(nch, a_ch * kk, NF).transpose(2, 0, 1)
            )
            blk = dreT[:, ecoff : ecoff + nch * CH].reshape(NG, nch, CH)
            blk[:, :, : a_ch * kk] = (
                dre.reshape(nch, a_ch * kk, NG).transpose(2, 0, 1)
            )
            slot[atoms] = zoff + np.arange(n)
            ecoff += nch * CH
            zoff += n_pad

        slots.append(slot)
        in_maps.append({**common, "ygT": ygT, "dreT": dreT})
    return in_maps, sig, slots


def kernel(**inputs) -> np.ndarray:
    from concourse.bass_utils import run_bass_kernel_spmd

    _check_b2(inputs["b2"])
    in_maps, sig, slots = make_in_maps(**inputs)
    nc = _get_nc(sig)
    res = run_bass_kernel_spmd(nc, in_maps, list(range(NCORES)))
    outs = []
    for m in range(NCORES):
        outT = np.asarray(res.results[m]["outT"])       # [NOUT, a_pad]
        outs.append(np.ascontiguousarray(outT.T[slots[m]]))
    return np.concatenate(outs, axis=0)


# b2 handling note: reference adds b2 after the second filter matmul.  In this
# problem b2 == 0; the general case would fold b2 into the product stage (the
# extra term is b2[h] * sum_k yg[i,k,h], computable host-side).  Assert so a
# non-zero b2 cannot silently give wrong results.
def _check_b2(b2):
    assert np.all(np.asarray(b2) == 0.0), "kernel assumes b2 == 0"
